# revision 13
# baseline (speedup 1.0000x reference)
"""Trainium2 Bass kernel for nn_Loss_Synonymy.

reference:
    diff = S1 - S2                       # [B, 256]
    d    = sqrt(sum(diff^2, axis=-1))    # [B]
    t    = tanh(d)
    err  = where(score >= 0.8, relu(1 - t), relu(1 + t))
         = relu(1 + t - 2*m*t),  m = (score >= 0.8)
    out  = sum(err) / B

Data-parallel over 8 NeuronCores; each core streams its 32768-row shard
(2 x 32 MiB) from HBM and reduces it to a [128, 1] partial err sum that the
host combines.

Per-core pipeline (memory-bound: the 64 MiB stream at ~410 GB/s is the
floor, so everything else is engineered off the critical path):

- Row mapping is p-major: partition p owns rows p*256 .. p*256+255 of the
  shard, result column c <-> row p*256 + c. A tile is a column range
  [c0, c0+J): per partition one contiguous J KiB HBM run -> near-peak DMA
  efficiency, and the score vector loads as ONE dense [128, 256] DMA (1 KiB
  per partition) instead of thousands of tiny strided descriptors. Main
  tiles J=16 (2 MiB); the shard tapers to J=8,4,2,2 so almost no compute
  remains after the last HBM byte lands.
- One custom DVE op per tile computes the running sum of (a-b)^2 along the
  free dim (scan); chunk-boundary differences of the running sum give the 16
  per-row sums in one extra [128, J] subtract. This replaces the baseline's
  subtract + 16 per-chunk square-reduce ops (2 DVE ops/tile instead of 17
  ops across two engines) and leaves ScalarE completely free during the
  stream.
- Epilogue: sqrt on the finished 240 columns overlaps the stream; after the
  last tile only a [128,16] sqrt, one tanh table switch, tanh, and a second
  custom DVE op (relu(1 + t - 2*m*t) with accumulate) remain. Partial sums
  leave via one 512 B store.
"""

import numpy as np

import concourse.bass as bass
import concourse.tile as tile
from concourse import bacc, mybir
from concourse.bass_utils import run_bass_kernel_spmd

F32 = mybir.dt.float32
AF = mybir.ActivationFunctionType
ALU = mybir.AluOpType

B = 262144
D = 256
NCORES = 8
BL = B // NCORES          # 32768 rows per core
THRESH = 0.8
B_ENG = "sync"            # engine issuing the s2 stream DMAs
BUFS_IN = 4               # input pool double-buffer depth

# (J = rows per partition) per tile; sum(J)*128 == BL. Tapered tail so the
# post-stream critical path is one tiny scan + the epilogue; J <= 4 in the
# taper keeps the per-tile scan shorter than its own DMA slot, so VectorE
# never backlogs past the end of the stream.
TILE_JS = [16] * 15 + [4, 4, 4, 2, 2]
assert sum(TILE_JS) * 128 == BL
NCOLS = sum(TILE_JS)      # 256 result columns per partition
JMAX = max(TILE_JS)

_NC_CACHE = {}

# ---------------------------------------------------------------------------
# Custom DVE ops. Registration follows the documented recipe (define a DveOp,
# append to OPS) done in-process; the uops sha is pinned to this env's own
# lower() output. Falls back to stock ops if anything is unavailable.


def _register_custom_ops():
    from concourse import dve_ops
    from concourse.dve_spec import (
        Spec, Src0, Src1, C0, C2, One, sq, relu, scan, lower, AluOp, _has_src1,
    )
    from concourse.dve_uop import DveOpSpec

    def ref_sqdiff_scan(in0, in1, s0, s1, imm2):
        d = (in0.astype(np.float32) - in1.astype(np.float32)) ** 2
        return np.cumsum(d.reshape(d.shape[0], -1), axis=-1).astype(np.float32)

    def ref_synerr(in0, in1, s0, s1, imm2):
        t = in0.astype(np.float32)
        m = (in1.astype(np.float32) >= s0).astype(np.float32)
        b = np.maximum(1.0 + t - imm2 * m * t, 0.0).astype(np.float32)
        return b, b.reshape(b.shape[0], -1).sum(axis=-1, keepdims=True)

    specs = {
        "SQDIFF_SCAN_LOSS44": Spec(
            body=scan(AluOp.ADD, sq(Src0 - Src1)),
            reference=ref_sqdiff_scan,
        ),
        "SYNERR_ACC_LOSS44": Spec(
            body=relu(One + Src0 - (Src1 >= C0) * Src0 * C2),
            accum=AluOp.ADD,
            reference=ref_synerr,
        ),
    }
    ops = {}
    for name, spec in specs.items():
        existing = next((o for o in dve_ops.OPS if o.name == name), None)
        if existing is not None:
            ops[name] = existing
            continue
        op = dve_ops.DveOp(name, spec, False, {})
        row = dve_ops._CUSTOM_DVE_ROW_BASE + len(dve_ops.OPS)
        assert row < 0x20
        for ver in ("v3", "v4"):
            try:
                op.uops_sha[ver] = DveOpSpec(
                    name=name, opcode=row, uops=lower(spec, ver=ver),
                    rd1_en=_has_src1(spec),
                ).sha(ver)
            except Exception:
                pass
        dve_ops.OPS.append(op)
        dve_ops._SUB_OPCODE_FOR_NAME[name] = row
        dve_ops.CUSTOM_DVE_SPECS[name] = spec
        ops[name] = op
    return ops


try:
    _OPS = _register_custom_ops()
    SQDIFF_SCAN = _OPS["SQDIFF_SCAN_LOSS44"]
    SYNERR_ACC = _OPS["SYNERR_ACC_LOSS44"]
    USE_CUSTOM = True
except Exception:
    USE_CUSTOM = False


# ---------------------------------------------------------------------------


def _build_nc():
    nc = bacc.Bacc(
        "TRN2", target_bir_lowering=False, debug=False, num_devices=NCORES
    )

    s1 = nc.dram_tensor("s1", [BL, D], F32, kind="ExternalInput").ap()
    s2 = nc.dram_tensor("s2", [BL, D], F32, kind="ExternalInput").ap()
    score = nc.dram_tensor("score", [BL], F32, kind="ExternalInput").ap()
    # 512 B per partition: under that, the final store degrades to 4 B
    # read-modify-write descriptors whose completion receipt costs ~6 us.
    # Only column 0 is meaningful; the rest stays zero and the host sums
    # everything.
    partial = nc.dram_tensor("partial", [128, 128], F32, kind="ExternalOutput").ap()

    # Running-sum scratch: col 0 stays 0 (scan writes cols 1..J*D), so the
    # chunk-boundary subtract can read lo/hi with one layout. Untracked raw
    # sbuf; all readers/writers are on VectorE, so program order suffices.
    cum = nc.alloc_sbuf_tensor("cum", [128, JMAX * D + 1], F32).ap()
    # Discarded elementwise output of the epilogue accumulate op.
    scr = nc.alloc_sbuf_tensor("scr", [128, NCOLS], F32).ap()

    nt_a = len(TILE_JS) - 2      # tiles whose sqrt overlaps the stream
    ncols_a = sum(TILE_JS[:nt_a])

    with tile.TileContext(nc) as tc:
        with (
            tc.tile_pool(name="in1", bufs=BUFS_IN) as p_in1,
            tc.tile_pool(name="in2", bufs=BUFS_IN) as p_in2,
            tc.tile_pool(name="diff", bufs=2) as p_diff,
            tc.tile_pool(name="persist", bufs=1) as p_per,
        ):
            sumsq_a = p_per.tile([128, ncols_a], F32, tag="sumsq_a")
            sumsq_b = p_per.tile([128, NCOLS - ncols_a], F32, tag="sumsq_b")
            score_sb = p_per.tile([128, NCOLS], F32, tag="score_sb")
            dist = p_per.tile([128, NCOLS], F32, tag="dist")
            th = p_per.tile([128, NCOLS], F32, tag="th")
            err = p_per.tile([128, 128], F32, tag="err")
            nc.vector.memset(err[:], 0.0)

            if USE_CUSTOM:
                nc.vector.memset(cum[:, 0:1], 0.0)

            # One dense score load (1 KiB/partition) on the ACT HWDGE ring,
            # early and off the critical path (needed by the final
            # accumulate only).
            nc.scalar.dma_start(
                score_sb[:], score.rearrange("(p x) -> p x", p=128)
            )

            s1_v = s1.rearrange("(p x) d -> p x d", p=128)
            s2_v = s2.rearrange("(p x) d -> p x d", p=128)

            def rowsums(t, J, c0, a, b):
                ss = (sumsq_a, c0) if t < nt_a else (sumsq_b, c0 - ncols_a)
                dst = ss[0][:, ss[1] : ss[1] + J]
                if USE_CUSTOM:
                    nc.vector._custom_dve(
                        SQDIFF_SCAN,
                        out=cum[:, 1 : J * D + 1],
                        in0=a[:, 0 : J * D],
                        in1=b[:, 0 : J * D],
                    )
                    hi = cum[:, 1 : J * D + 1].rearrange(
                        "p (j d) -> p j d", d=D
                    )[:, :, D - 1 : D]
                    lo = cum[:, 0 : J * D].rearrange(
                        "p (j d) -> p j d", d=D
                    )[:, :, 0:1]
                    nc.vector.tensor_tensor(
                        dst.rearrange("p (j o) -> p j o", o=1),
                        hi, lo, ALU.subtract,
                    )
                else:
                    diff = p_diff.tile([128, JMAX * D], F32, tag="diff")
                    nc.vector.tensor_sub(
                        diff[:, 0 : J * D], a[:, 0 : J * D], b[:, 0 : J * D]
                    )
                    for j in range(J):
                        chunk = diff[:, j * D : (j + 1) * D]
                        col = dst[:, j : j + 1]
                        if j % 8 < 3:
                            nc.vector.scalar_tensor_tensor(
                                scr[:, 0:D], chunk, 1.0, chunk,
                                ALU.mult, ALU.mult, accum_out=col,
                            )
                        else:
                            nc.scalar.activation(
                                scr[:, 0:D], chunk, AF.Square, accum_out=col
                            )

            for t, J in enumerate(TILE_JS):
                c0 = sum(TILE_JS[:t])
                a = p_in1.tile([128, JMAX * D], F32, tag="a")
                nc.sync.dma_start(
                    a[:, 0 : J * D].rearrange("p (j d) -> p j d", d=D),
                    s1_v[:, c0 : c0 + J, :],
                )
                b = p_in2.tile([128, JMAX * D], F32, tag="b")
                getattr(nc, B_ENG).dma_start(
                    b[:, 0 : J * D].rearrange("p (j d) -> p j d", d=D),
                    s2_v[:, c0 : c0 + J, :],
                )
                rowsums(t, J, c0, a, b)

                if t == nt_a - 1:
                    # Bulk sqrt overlaps the remaining stream; the tanh
                    # table switch that follows it does too.
                    nc.scalar.activation(
                        dist[:, 0:ncols_a], sumsq_a[:], AF.Sqrt
                    )

            nc.scalar.activation(dist[:, ncols_a:NCOLS], sumsq_b[:], AF.Sqrt)
            nc.scalar.activation(th[:], dist[:], AF.Tanh)

            if USE_CUSTOM:
                nc.vector._custom_dve(
                    SYNERR_ACC,
                    out=scr[:],
                    in0=th[:],
                    in1=score_sb[:],
                    s0=THRESH,
                    imm2=2.0,
                    accum_out=err[:, 0:1],
                )
            else:
                sgn = p_per.tile([128, NCOLS], F32, tag="sgn")
                nc.vector.tensor_scalar(
                    sgn[:], score_sb[:], THRESH, -2.0, ALU.is_ge, ALU.mult
                )
                st = p_per.tile([128, NCOLS], F32, tag="st")
                nc.vector.scalar_tensor_tensor(
                    st[:], sgn[:], 1.0, th[:], ALU.add, ALU.mult
                )
                nc.scalar.activation(
                    scr[:], st[:], AF.Relu, bias=1.0, scale=1.0,
                    accum_out=err[:, 0:1],
                )

            nc.sync.dma_start(partial, err[:])

    nc.compile()
    return nc


def _get_nc():
    if "nc" not in _NC_CACHE:
        _NC_CACHE["nc"] = _build_nc()
    return _NC_CACHE["nc"]


def make_in_maps(S1_out, S2_out, synonymy_score):
    in_maps = []
    for c in range(NCORES):
        lo, hi = c * BL, (c + 1) * BL
        in_maps.append(
            {
                "s1": np.ascontiguousarray(S1_out[lo:hi], dtype=np.float32),
                "s2": np.ascontiguousarray(S2_out[lo:hi], dtype=np.float32),
                "score": np.ascontiguousarray(
                    synonymy_score[lo:hi], dtype=np.float32
                ),
            }
        )
    return in_maps


def combine(results):
    total = np.float64(0.0)
    for r in results:
        total += r["partial"].astype(np.float64).sum()
    return np.asarray(total / B, dtype=np.float32)


def run(S1_out, S2_out, synonymy_score, trace=False, **trace_kwargs):
    nc = _get_nc()
    in_maps = make_in_maps(S1_out, S2_out, synonymy_score)
    res = run_bass_kernel_spmd(
        nc, in_maps, list(range(NCORES)), trace=trace, **trace_kwargs
    )
    return combine(res.results), res


def kernel(S1_out, S2_out, synonymy_score):
    out, _ = run(S1_out, S2_out, synonymy_score)
    return out
